# revision 17
# baseline (speedup 1.0000x reference)
"""Trainium2 Bass kernel for hash-indexed gather + GEMM (gnn_message_passing).

Reference computation:
    g[b, c, hw, k] = x.reshape(B, C*H*W)[b, hash_idx[c, hw, k]]
    out[kp, b*hw]  = weights[kp, c*k] @ g  (contraction over ck = 288)
    returns [B, KP, H, W]

Strategy (8 NeuronCores, no cross-core communication):
  - Host transposes x to xT[C*H*W, B] float32 so one gathered "row" is the
    value of one (c,pixel) across all 64 images = 256 contiguous bytes.
  - Each core owns 128 of the 1024 output pixels (all images, all channels).
  - On-device dma_gather (SWDGE) gathers rows straight from HBM and writes
    row i to SBUF partition i%128; we order the index list so partitions
    line up with the GEMM contraction dim (ck), i.e. the gather performs the
    im2col transpose for free.
  - 3 ck-chunks (0:128, 128:256, 256:288) accumulate into PSUM. The 32-wide
    third chunk is packed 4x along partitions covering hw%4 classes.
  - Output per core: [KP=64, 8192] = (m4, q, b) column order, reassembled on
    host.
"""

import numpy as np

B, C, H, W = 64, 32, 32, 32
K = 9
KP = 64
HWT = H * W          # 1024
CHW = C * H * W      # 32768
CK = C * K           # 288
NCORES = 8
HWC = HWT // NCORES  # 128 pixels per core
NCOLS = HWC * B      # 8192 output columns per core

_NC_CACHE = {}


def _build_nc():
    """Build the (single-program SPMD) Bass graph; all per-core variation is
    carried by the input data."""
    from concourse import bacc, bass, library_config, mybir, tile

    f32 = mybir.dt.float32
    f32r = mybir.dt.float32r
    i16 = mybir.dt.int16

    nc = bacc.Bacc(num_swdge_queues=4)

    xt = nc.declare_dram_parameter("xt", [CHW, B], f32, isOutput=False)
    idx = nc.declare_dram_parameter("idx", [128, 2304], i16, isOutput=False)
    w = nc.declare_dram_parameter("w", [128, 256], f32, isOutput=False)
    out = nc.declare_dram_parameter("out", [KP, NCOLS], f32, isOutput=True)

    # Load the GPSIMD library before the TileContext preamble so the ~10us
    # IRAM load overlaps the framework's start-of-block barriers.
    nc.gpsimd.load_library(library_config.mlp)

    with tile.TileContext(nc) as tc:
        with (
            tc.tile_pool(name="const", bufs=1) as const_pool,
            tc.tile_pool(name="g", bufs=1) as g_pool,
            tc.tile_pool(name="stage", bufs=1) as stage_pool,
            tc.tile_pool(name="psum", bufs=6, space="PSUM") as psum_pool,
        ):

            idx_sb = const_pool.tile([128, 2304], i16)
            nc.sync.dma_start(out=idx_sb[:], in_=idx[:])
            # float32r tiles: same bits as f32, but lets the fp32r matmuls
            # (1 cycle/row vs 4 for fp32) pass BIR verification.
            w_sb = const_pool.tile([128, 256], f32r)
            nc.sync.dma_start(out=w_sb[:], in_=w[:].bitcast(f32r))

            # Gather targets as per-subcall tiles so Tile's dependency
            # tracking lets supertile s start as soon as ITS slices landed.
            # 2048-idx calls (16 slots / 2 supertiles each) amortize the
            # ~550ns fixed SWDGE cost; the tail calls are 1024-idx so the
            # last round finishes sooner. Schedule: 5 calls per queue
            # (4x2048 + 1x1024 = 9216 rows), one queue per GPSIMD cpu pair.
            # g0/g1 coverage in 128-row "slots" (slot j = pixel-slot j, 8 idx
            # cols each). Head/tail rounds are 512-row calls: the first call
            # paces the SWDGE pipeline fill (lag = its duration) and the last
            # round sets the drain tail, so both are small.
            g0a = [g_pool.tile([128, 4, B], f32r, tag=f"g0a_{i}",
                               name=f"g0a_{i}") for i in range(2)]
            g1a = [g_pool.tile([128, 4, B], f32r, tag=f"g1a_{i}",
                               name=f"g1a_{i}") for i in range(2)]
            g0b = [g_pool.tile([128, 16, B], f32r, tag=f"g0b_{t}",
                               name=f"g0b_{t}") for t in range(7)]
            g1b = [g_pool.tile([128, 16, B], f32r, tag=f"g1b_{t}",
                               name=f"g1b_{t}") for t in range(7)]
            g2p = [g_pool.tile([128, 16, B], f32r, tag=f"g2p_{v}",
                               name=f"g2p_{v}") for v in range(2)]
            g0c = [g_pool.tile([128, 4, B], f32r, tag=f"g0c_{i}",
                               name=f"g0c_{i}") for i in range(2)]
            g1c = [g_pool.tile([128, 4, B], f32r, tag=f"g1c_{i}",
                               name=f"g1c_{i}") for i in range(2)]

            nidx_regs = {512: nc.gpsimd.to_reg(512),
                         2048: nc.gpsimd.to_reg(2048)}

            def gather(dst, col0, q, nidx):
                nc.gpsimd.dma_gather(
                    dst[:], xt[:].bitcast(f32r),
                    idx_sb[:, col0:col0 + nidx // 16], nidx,
                    nidx_regs[nidx], B, queue_num=q, single_packet=False,
                )

            # g0 slot a -> idx col 8a; g1 -> 1024+8a; g2 slot-unit j -> 2048+8j.
            # Per queue: [512, 2048 x4, 512] = 9216 rows. b-tile slot windows:
            # q0 owns g0b t=0,2,4,6 (slots 8:24,40:56,72:88,104:120), q2 owns
            # t=1,3,5 (24:40,56:72,88:104) + g2[0:16]; q1/q3 mirror for g1.
            gather(g0a[0], 0, 0, 512)          # r1: g0 slots 0:4
            gather(g1a[0], 1024 + 0, 1, 512)   # r1: g1 slots 0:4
            gather(g0a[1], 32, 2, 512)         # r1: g0 slots 4:8
            gather(g1a[1], 1024 + 32, 3, 512)  # r1: g1 slots 4:8
            gather(g0b[0], 8 * 8, 0, 2048)     # r2: g0 8:24
            gather(g1b[0], 1024 + 8 * 8, 1, 2048)
            gather(g2p[0], 2048 + 0, 2, 2048)  # r2: g2 classes u=0,1
            gather(g2p[1], 2048 + 128, 3, 2048)
            for rnd in range(3):
                tq0, tq2 = 2 + 2 * rnd, 1 + 2 * rnd
                a0, a2 = 8 + 16 * tq0, 8 + 16 * tq2
                gather(g0b[tq0], 8 * a0, 0, 2048)
                gather(g1b[tq0], 1024 + 8 * a0, 1, 2048)
                gather(g0b[tq2], 8 * a2, 2, 2048)
                gather(g1b[tq2], 1024 + 8 * a2, 3, 2048)
            gather(g0c[0], 8 * 120, 0, 512)    # r6: g0 120:124
            gather(g1c[0], 1024 + 8 * 120, 1, 512)
            gather(g0c[1], 8 * 124, 2, 512)    # r6: g0 124:128
            gather(g1c[1], 1024 + 8 * 124, 3, 512)

            stage_t = [stage_pool.tile([KP, 1024], f32, tag=f"st_{r}",
                                       name=f"st_{r}")
                       for r in range(8)]

            def mm(ps, stat, mov, start, stop):
                # float32r: full-rate (1 cycle/row) fp32 matmul on trn2.
                nc.tensor.matmul(ps, stat, mov, start=start, stop=stop)

            def mm_chunk2(ps, m4, g2tile, s0, s1):
                # chunk2: partitions 32*m4..+32 of g2 (AP base must be
                # 0/32/64; the base-96 block runs as K=64 at base 64 with
                # zeroed weight rows for partitions 64..95).
                if m4 < 3:
                    mm(ps, w_sb[32 * m4:32 * m4 + 32, 128:192],
                       g2tile[32 * m4:32 * m4 + 32, s0:s1, :],
                       start=False, stop=True)
                else:
                    mm(ps, w_sb[64:128, 192:256],
                       g2tile[64:128, s0:s1, :], start=False, stop=True)

            # 16 supertiles of 512 output columns; supertile s: m4 = s//4
            # (hw%4 class), q-range 8*(s%4)..+8 (hw//4). s=0/15 span two
            # 4-slot head/tail tiles, so their chunks run as two 256-col
            # matmul groups into disjoint PSUM column halves.
            for s in range(16):
                m4, u = s // 4, s % 4
                o2 = 8 * (u % 2)
                ps = psum_pool.tile([KP, 512], f32)
                if s in (0, 15):
                    pair = (g0a, g1a) if s == 0 else (g0c, g1c)
                    for i in range(2):
                        psl = ps[:, 256 * i:256 * (i + 1)]
                        mm(psl, w_sb[:, 0:64], pair[0][i][:],
                           start=True, stop=False)
                        mm(psl, w_sb[:, 64:128], pair[1][i][:],
                           start=False, stop=False)
                        mm_chunk2(psl, m4, g2p[u // 2],
                                  o2 + 4 * i, o2 + 4 * i + 4)
                else:
                    t, half = (s - 1) // 2, (s - 1) % 2
                    o = 8 * half
                    mm(ps[:], w_sb[:, 0:64], g0b[t][:, o:o + 8, :],
                       start=True, stop=False)
                    mm(ps[:], w_sb[:, 64:128], g1b[t][:, o:o + 8, :],
                       start=False, stop=False)
                    mm_chunk2(ps[:], m4, g2p[u // 2], o2, o2 + 8)
                eng = nc.vector.tensor_copy if s % 2 == 0 else nc.scalar.copy
                eng(out=stage_t[s // 2][:, 512 * (s % 2):512 * (s % 2 + 1)],
                    in_=ps[:])
                if s % 2 == 1:
                    nc.sync.dma_start(
                        out=out[:, 1024 * (s // 2):1024 * (s // 2 + 1)],
                        in_=stage_t[s // 2][:],
                    )

    nc.finalize()
    return nc


def get_nc():
    if "nc" not in _NC_CACHE:
        _NC_CACHE["nc"] = _build_nc()
    return _NC_CACHE["nc"]


def _wrap_idx(flat):
    """SWDGE index layout: unwrapped[i] = wrapped[i % 16, i // 16], replicated
    across the 8 GPSIMD 16-partition groups."""
    n = flat.shape[0]
    wrapped = flat.reshape(n // 16, 16).T  # [16, n/16]
    return np.tile(wrapped, (8, 1))        # [128, n/16]


def make_inputs(x, hash_idx, weights):
    """Host-side sharding/layout prep. Returns in_maps for the 8 cores."""
    x = np.asarray(x, dtype=np.float32)
    hash_idx = np.asarray(hash_idx)
    weights = np.asarray(weights, dtype=np.float32)

    xtr = np.ascontiguousarray(x.reshape(B, CHW).T)  # [CHW, B] f32

    # idxmat[ck, hw] with ck = c*9 + k
    idxmat = (
        hash_idx.reshape(C, HWT, K).transpose(0, 2, 1).reshape(CK, HWT)
    ).astype(np.int16)

    # weights, stationary layout: [contraction partitions, 64 kp]
    w_sb = np.zeros((128, 256), dtype=np.float32)
    w_sb[:, 0:64] = weights[:, 0:128].T
    w_sb[:, 64:128] = weights[:, 128:256].T
    w_sb[:, 128:192] = np.tile(weights[:, 256:288].T, (4, 1))
    # m4=3 special stationary: rows 64..95 (m4=2 data) zero, 96..127 real
    w_sb[96:128, 192:256] = weights[:, 256:288].T

    # column order inside a core: col = m4*2048 + q*64 + b ; hw_local = 4q+m4
    j_arange = np.arange(HWC)
    hw_of_slot = 4 * (j_arange % 32) + j_arange // 32  # slot j -> hw_local

    in_maps = []
    for m in range(NCORES):
        sub = idxmat[:, m * HWC:(m + 1) * HWC]  # [288, 128] int16
        # chunks 0/1: i = j*128 + p (slot-major, ck-local minor)
        c0 = np.ascontiguousarray(sub[0:128, hw_of_slot].T).reshape(-1)
        c1 = np.ascontiguousarray(sub[128:256, hw_of_slot].T).reshape(-1)
        # chunk 2: i = q*128 + m4*32 + ck_local ; partition = 32*m4+ck
        #   value  = idx[256+ck, hw = 4q+m4]
        sub2 = sub[256:288, :]                     # [32, 128]
        arr2 = sub2.T.reshape(32, 4, 32)           # [q, m4, ck]
        c2 = np.ascontiguousarray(arr2).reshape(-1)
        idx_all = np.concatenate(
            [_wrap_idx(c0), _wrap_idx(c1), _wrap_idx(c2)], axis=1
        )
        idx_all = np.ascontiguousarray(idx_all, dtype=np.int16)
        in_maps.append({"xt": xtr, "idx": idx_all, "w": w_sb})
    return in_maps


def assemble_output(shards):
    """shards[m]: [KP, 8192] in (m4, q, b) column order -> [B, KP, H, W]."""
    out = np.empty((B, KP, HWT), dtype=np.float32)
    for m in range(NCORES):
        sh = np.asarray(shards[m]).reshape(KP, 4, 32, B)  # [kp, m4, q, b]
        blk = sh.transpose(3, 0, 2, 1).reshape(B, KP, HWC)  # hw = 4q + m4
        out[:, :, m * HWC:(m + 1) * HWC] = blk
    return out.reshape(B, KP, H, W)


def kernel(x, hash_idx, weights):
    import time

    from concourse.bass_utils import run_bass_kernel_spmd

    in_maps = make_inputs(x, hash_idx, weights)
    last_err = None
    for attempt in range(4):
        try:
            nc = get_nc()
            res = run_bass_kernel_spmd(nc, in_maps, list(range(NCORES)))
            shards = [res.results[m]["out"] for m in range(NCORES)]
            return assemble_output(shards)
        except Exception as e:  # transient NRT/device errors — retry
            last_err = e
            _NC_CACHE.clear()  # rebuild graph/executable on retry
            time.sleep(5.0 * (attempt + 1))
    raise last_err



# revision 19
# speedup vs baseline: 1.0487x; 1.0487x over previous
"""Trainium2 Bass kernel for hash-indexed gather + GEMM (gnn_message_passing).

Reference computation:
    g[b, c, hw, k] = x.reshape(B, C*H*W)[b, hash_idx[c, hw, k]]
    out[kp, b*hw]  = weights[kp, c*k] @ g  (contraction over ck = 288)
    returns [B, KP, H, W]

Strategy (8 NeuronCores, no cross-core communication):
  - Host transposes x to xT[C*H*W, B] float32 so one gathered "row" is the
    value of one (c,pixel) across all 64 images = 256 contiguous bytes.
  - Each core owns 128 of the 1024 output pixels (all images, all channels).
  - On-device dma_gather (SWDGE) gathers rows straight from HBM and writes
    row i to SBUF partition i%128; we order the index list so partitions
    line up with the GEMM contraction dim (ck), i.e. the gather performs the
    im2col transpose for free.
  - 3 ck-chunks (0:128, 128:256, 256:288) accumulate into PSUM. The 32-wide
    third chunk is packed 4x along partitions covering hw%4 classes.
  - Output per core: [KP=64, 8192] = (m4, q, b) column order, reassembled on
    host.
"""

import numpy as np

B, C, H, W = 64, 32, 32, 32
K = 9
KP = 64
HWT = H * W          # 1024
CHW = C * H * W      # 32768
CK = C * K           # 288
NCORES = 8
HWC = HWT // NCORES  # 128 pixels per core
NCOLS = HWC * B      # 8192 output columns per core

_NC_CACHE = {}


def _build_nc():
    """Build the (single-program SPMD) Bass graph; all per-core variation is
    carried by the input data."""
    from concourse import bacc, bass, library_config, mybir, tile

    f32 = mybir.dt.float32
    f32r = mybir.dt.float32r
    i16 = mybir.dt.int16

    nc = bacc.Bacc(num_swdge_queues=4)

    xt = nc.declare_dram_parameter("xt", [CHW, B], f32, isOutput=False)
    idx = nc.declare_dram_parameter("idx", [128, 2304], i16, isOutput=False)
    w = nc.declare_dram_parameter("w", [128, 256], f32, isOutput=False)
    out = nc.declare_dram_parameter("out", [KP, NCOLS], f32, isOutput=True)

    # Load the GPSIMD library before the TileContext preamble so the ~10us
    # IRAM load overlaps the framework's start-of-block barriers.
    nc.gpsimd.load_library(library_config.mlp)

    with tile.TileContext(nc) as tc:
        with (
            tc.tile_pool(name="const", bufs=1) as const_pool,
            tc.tile_pool(name="g", bufs=1) as g_pool,
            tc.tile_pool(name="stage", bufs=1) as stage_pool,
            tc.tile_pool(name="psum", bufs=6, space="PSUM") as psum_pool,
        ):

            idx_sb = const_pool.tile([128, 2304], i16)
            nc.sync.dma_start(out=idx_sb[:], in_=idx[:])
            # float32r tiles: same bits as f32, but lets the fp32r matmuls
            # (1 cycle/row vs 4 for fp32) pass BIR verification.
            w_sb = const_pool.tile([128, 256], f32r)
            nc.sync.dma_start(out=w_sb[:], in_=w[:].bitcast(f32r))

            # Gather targets as per-subcall tiles so Tile's dependency
            # tracking lets supertile s start as soon as ITS slices landed.
            # 2048-idx calls (16 slots / 2 supertiles each) amortize the
            # ~550ns fixed SWDGE cost; the tail calls are 1024-idx so the
            # last round finishes sooner. Schedule: 5 calls per queue
            # (4x2048 + 1x1024 = 9216 rows), one queue per GPSIMD cpu pair.
            # Uniform 1024-row calls, 9 per queue (36 total), strict
            # round-robin. The SWDGE command pipe admits ~3 in-flight calls,
            # so rounds self-pace at one call-duration each; uniform sizing
            # minimizes the (n+1)/n pipeline overhead. Round 1 is the four
            # g2 calls (shared by every supertile class); rounds 2-9 stream
            # the (g0, g1) pair for two supertiles each.
            g0t = [g_pool.tile([128, 8, B], f32r, tag=f"g0_{s}",
                               name=f"g0_{s}") for s in range(16)]
            g1t = [g_pool.tile([128, 8, B], f32r, tag=f"g1_{s}",
                               name=f"g1_{s}") for s in range(16)]
            g2t = [g_pool.tile([128, 8, B], f32r, tag=f"g2_{u}",
                               name=f"g2_{u}") for u in range(4)]

            nidx_reg = nc.gpsimd.to_reg(1024)
            qi = [0]

            def gather(dst, col0):
                nc.gpsimd.dma_gather(
                    dst[:], xt[:].bitcast(f32r),
                    idx_sb[:, col0:col0 + 64], 1024,
                    nidx_reg, B, queue_num=qi[0] % 4, single_packet=False,
                )
                qi[0] += 1

            # idx cols: g0 supertile s -> 64s; g1 -> 1024+64s; g2 class u
            # -> 2048+64u.
            for u in range(4):
                gather(g2t[u], 2048 + 64 * u)
            for s in range(16):
                gather(g0t[s], 64 * s)
                gather(g1t[s], 1024 + 64 * s)

            stage_t = [stage_pool.tile([KP, 1024], f32, tag=f"st_{r}",
                                       name=f"st_{r}")
                       for r in range(8)]

            def mm(ps, stat, mov, start, stop):
                # float32r: full-rate (1 cycle/row) fp32 matmul on trn2.
                nc.tensor.matmul(ps, stat, mov, start=start, stop=stop)

            def mm_chunk2(ps, m4, g2tile, s0, s1):
                # chunk2: partitions 32*m4..+32 of g2 (AP base must be
                # 0/32/64; the base-96 block runs as K=64 at base 64 with
                # zeroed weight rows for partitions 64..95).
                if m4 < 3:
                    mm(ps, w_sb[32 * m4:32 * m4 + 32, 128:192],
                       g2tile[32 * m4:32 * m4 + 32, s0:s1, :],
                       start=False, stop=True)
                else:
                    mm(ps, w_sb[64:128, 192:256],
                       g2tile[64:128, s0:s1, :], start=False, stop=True)

            # 16 supertiles of 512 output columns; supertile s: m4 = s//4
            # (hw%4 class), q-range 8*(s%4)..+8 (hw//4).
            for s in range(16):
                m4, u = s // 4, s % 4
                ps = psum_pool.tile([KP, 512], f32)
                mm(ps[:], w_sb[:, 0:64], g0t[s][:], start=True, stop=False)
                mm(ps[:], w_sb[:, 64:128], g1t[s][:], start=False, stop=False)
                mm_chunk2(ps[:], m4, g2t[u], 0, 8)
                eng = nc.vector.tensor_copy if s % 2 == 0 else nc.scalar.copy
                eng(out=stage_t[s // 2][:, 512 * (s % 2):512 * (s % 2 + 1)],
                    in_=ps[:])
                if s % 2 == 1:
                    nc.sync.dma_start(
                        out=out[:, 1024 * (s // 2):1024 * (s // 2 + 1)],
                        in_=stage_t[s // 2][:],
                    )

    nc.finalize()
    return nc


def get_nc():
    if "nc" not in _NC_CACHE:
        _NC_CACHE["nc"] = _build_nc()
    return _NC_CACHE["nc"]


def _wrap_idx(flat):
    """SWDGE index layout: unwrapped[i] = wrapped[i % 16, i // 16], replicated
    across the 8 GPSIMD 16-partition groups."""
    n = flat.shape[0]
    wrapped = flat.reshape(n // 16, 16).T  # [16, n/16]
    return np.tile(wrapped, (8, 1))        # [128, n/16]


def make_inputs(x, hash_idx, weights):
    """Host-side sharding/layout prep. Returns in_maps for the 8 cores."""
    x = np.asarray(x, dtype=np.float32)
    hash_idx = np.asarray(hash_idx)
    weights = np.asarray(weights, dtype=np.float32)

    xtr = np.ascontiguousarray(x.reshape(B, CHW).T)  # [CHW, B] f32

    # idxmat[ck, hw] with ck = c*9 + k
    idxmat = (
        hash_idx.reshape(C, HWT, K).transpose(0, 2, 1).reshape(CK, HWT)
    ).astype(np.int16)

    # weights, stationary layout: [contraction partitions, 64 kp]
    w_sb = np.zeros((128, 256), dtype=np.float32)
    w_sb[:, 0:64] = weights[:, 0:128].T
    w_sb[:, 64:128] = weights[:, 128:256].T
    w_sb[:, 128:192] = np.tile(weights[:, 256:288].T, (4, 1))
    # m4=3 special stationary: rows 64..95 (m4=2 data) zero, 96..127 real
    w_sb[96:128, 192:256] = weights[:, 256:288].T

    # column order inside a core: col = m4*2048 + q*64 + b ; hw_local = 4q+m4
    j_arange = np.arange(HWC)
    hw_of_slot = 4 * (j_arange % 32) + j_arange // 32  # slot j -> hw_local

    in_maps = []
    for m in range(NCORES):
        sub = idxmat[:, m * HWC:(m + 1) * HWC]  # [288, 128] int16
        # chunks 0/1: i = j*128 + p (slot-major, ck-local minor)
        c0 = np.ascontiguousarray(sub[0:128, hw_of_slot].T).reshape(-1)
        c1 = np.ascontiguousarray(sub[128:256, hw_of_slot].T).reshape(-1)
        # chunk 2: i = q*128 + m4*32 + ck_local ; partition = 32*m4+ck
        #   value  = idx[256+ck, hw = 4q+m4]
        sub2 = sub[256:288, :]                     # [32, 128]
        arr2 = sub2.T.reshape(32, 4, 32)           # [q, m4, ck]
        c2 = np.ascontiguousarray(arr2).reshape(-1)
        idx_all = np.concatenate(
            [_wrap_idx(c0), _wrap_idx(c1), _wrap_idx(c2)], axis=1
        )
        idx_all = np.ascontiguousarray(idx_all, dtype=np.int16)
        in_maps.append({"xt": xtr, "idx": idx_all, "w": w_sb})
    return in_maps


def assemble_output(shards):
    """shards[m]: [KP, 8192] in (m4, q, b) column order -> [B, KP, H, W]."""
    out = np.empty((B, KP, HWT), dtype=np.float32)
    for m in range(NCORES):
        sh = np.asarray(shards[m]).reshape(KP, 4, 32, B)  # [kp, m4, q, b]
        blk = sh.transpose(3, 0, 2, 1).reshape(B, KP, HWC)  # hw = 4q + m4
        out[:, :, m * HWC:(m + 1) * HWC] = blk
    return out.reshape(B, KP, H, W)


def kernel(x, hash_idx, weights):
    import time

    from concourse.bass_utils import run_bass_kernel_spmd

    in_maps = make_inputs(x, hash_idx, weights)
    last_err = None
    for attempt in range(4):
        try:
            nc = get_nc()
            res = run_bass_kernel_spmd(nc, in_maps, list(range(NCORES)))
            shards = [res.results[m]["out"] for m in range(NCORES)]
            return assemble_output(shards)
        except Exception as e:  # transient NRT/device errors — retry
            last_err = e
            _NC_CACHE.clear()  # rebuild graph/executable on retry
            time.sleep(5.0 * (attempt + 1))
    raise last_err



# revision 20
# speedup vs baseline: 1.0535x; 1.0047x over previous
"""Trainium2 Bass kernel for hash-indexed gather + GEMM (gnn_message_passing).

Reference computation:
    g[b, c, hw, k] = x.reshape(B, C*H*W)[b, hash_idx[c, hw, k]]
    out[kp, b*hw]  = weights[kp, c*k] @ g  (contraction over ck = 288)
    returns [B, KP, H, W]

Strategy (8 NeuronCores, no cross-core communication):
  - Host transposes x to xT[C*H*W, B] float32 so one gathered "row" is the
    value of one (c,pixel) across all 64 images = 256 contiguous bytes.
  - Each core owns 128 of the 1024 output pixels (all images, all channels).
  - On-device dma_gather (SWDGE) gathers rows straight from HBM and writes
    row i to SBUF partition i%128; we order the index list so partitions
    line up with the GEMM contraction dim (ck), i.e. the gather performs the
    im2col transpose for free.
  - 3 ck-chunks (0:128, 128:256, 256:288) accumulate into PSUM. The 32-wide
    third chunk is packed 4x along partitions covering hw%4 classes.
  - Output per core: [KP=64, 8192] = (m4, q, b) column order, reassembled on
    host.
"""

import numpy as np

B, C, H, W = 64, 32, 32, 32
K = 9
KP = 64
HWT = H * W          # 1024
CHW = C * H * W      # 32768
CK = C * K           # 288
NCORES = 8
HWC = HWT // NCORES  # 128 pixels per core
NCOLS = HWC * B      # 8192 output columns per core

_NC_CACHE = {}


def _build_nc():
    """Build the (single-program SPMD) Bass graph; all per-core variation is
    carried by the input data."""
    from concourse import bacc, bass, library_config, mybir, tile

    f32 = mybir.dt.float32
    f32r = mybir.dt.float32r
    i16 = mybir.dt.int16

    nc = bacc.Bacc(num_swdge_queues=4)

    xt = nc.declare_dram_parameter("xt", [CHW, B], f32, isOutput=False)
    idx = nc.declare_dram_parameter("idx", [128, 2304], i16, isOutput=False)
    w = nc.declare_dram_parameter("w", [128, 256], f32, isOutput=False)
    out = nc.declare_dram_parameter("out", [KP, NCOLS], f32, isOutput=True)

    # Load the GPSIMD library before the TileContext preamble so the ~10us
    # IRAM load overlaps the framework's start-of-block barriers.
    nc.gpsimd.load_library(library_config.mlp)

    with tile.TileContext(nc) as tc:
        with (
            tc.tile_pool(name="const", bufs=1) as const_pool,
            tc.tile_pool(name="g", bufs=1) as g_pool,
            tc.tile_pool(name="stage", bufs=1) as stage_pool,
            tc.tile_pool(name="psum", bufs=6, space="PSUM") as psum_pool,
        ):

            idx_sb = const_pool.tile([128, 2304], i16)
            nc.sync.dma_start(out=idx_sb[:], in_=idx[:])
            # float32r tiles: same bits as f32, but lets the fp32r matmuls
            # (1 cycle/row vs 4 for fp32) pass BIR verification.
            w_sb = const_pool.tile([128, 256], f32r)
            nc.sync.dma_start(out=w_sb[:], in_=w[:].bitcast(f32r))

            # Gather targets as per-subcall tiles so Tile's dependency
            # tracking lets supertile s start as soon as ITS slices landed.
            # 2048-idx calls (16 slots / 2 supertiles each) amortize the
            # ~550ns fixed SWDGE cost; the tail calls are 1024-idx so the
            # last round finishes sooner. Schedule: 5 calls per queue
            # (4x2048 + 1x1024 = 9216 rows), one queue per GPSIMD cpu pair.
            # Uniform 1024-row calls, 9 per queue (36 total), strict
            # round-robin. The SWDGE command pipe admits ~3 in-flight calls,
            # so rounds self-pace at one call-duration each; uniform sizing
            # minimizes the (n+1)/n pipeline overhead. Round 1 is the four
            # g2 calls (shared by every supertile class); rounds 2-9 stream
            # the (g0, g1) pair for two supertiles each.
            g0t = [g_pool.tile([128, 8, B], f32r, tag=f"g0_{s}",
                               name=f"g0_{s}") for s in range(16)]
            g1t = [g_pool.tile([128, 8, B], f32r, tag=f"g1_{s}",
                               name=f"g1_{s}") for s in range(16)]
            g2t = [g_pool.tile([128, 8, B], f32r, tag=f"g2_{u}",
                               name=f"g2_{u}") for u in range(4)]

            nidx_reg = nc.gpsimd.to_reg(1024)
            qi = [0]

            def gather(dst, col0):
                nc.gpsimd.dma_gather(
                    dst[:], xt[:].bitcast(f32r),
                    idx_sb[:, col0:col0 + 64], 1024,
                    nidx_reg, B, queue_num=qi[0] % 4, single_packet=False,
                )
                qi[0] += 1

            # idx cols: g0 supertile s -> 64s; g1 -> 1024+64s; g2 class u
            # -> 2048+64u.
            for u in range(4):
                gather(g2t[u], 2048 + 64 * u)
            for s in range(16):
                gather(g0t[s], 64 * s)
                gather(g1t[s], 1024 + 64 * s)

            stage_t = [stage_pool.tile([KP, 1024], f32, tag=f"st_{r}",
                                       name=f"st_{r}")
                       for r in range(8)]

            def mm(ps, stat, mov, start, stop):
                # float32r: full-rate (1 cycle/row) fp32 matmul on trn2.
                nc.tensor.matmul(ps, stat, mov, start=start, stop=stop)

            def mm_chunk2(ps, m4, g2tile, s0, s1):
                # chunk2: partitions 32*m4..+32 of g2 (AP base must be
                # 0/32/64; the base-96 block runs as K=64 at base 64 with
                # zeroed weight rows for partitions 64..95).
                if m4 < 3:
                    mm(ps, w_sb[32 * m4:32 * m4 + 32, 128:192],
                       g2tile[32 * m4:32 * m4 + 32, s0:s1, :],
                       start=False, stop=True)
                else:
                    mm(ps, w_sb[64:128, 192:256],
                       g2tile[64:128, s0:s1, :], start=False, stop=True)

            # 16 supertiles of 512 output columns; supertile s: m4 = s//4
            # (hw%4 class), q-range 8*(s%4)..+8 (hw//4).
            for s in range(16):
                m4, u = s // 4, s % 4
                ps = psum_pool.tile([KP, 512], f32)
                mm(ps[:], w_sb[:, 0:64], g0t[s][:], start=True, stop=False)
                mm(ps[:], w_sb[:, 64:128], g1t[s][:], start=False, stop=False)
                mm_chunk2(ps[:], m4, g2t[u], 0, 8)
                eng = nc.vector.tensor_copy if s % 2 == 0 else nc.scalar.copy
                eng(out=stage_t[s // 2][:, 512 * (s % 2):512 * (s % 2 + 1)],
                    in_=ps[:])
                if s % 2 == 1:
                    nc.sync.dma_start(
                        out=out[:, 1024 * (s // 2):1024 * (s // 2 + 1)],
                        in_=stage_t[s // 2][:],
                    )

    nc.finalize()
    _hoist_preamble(nc)
    return nc


def _hoist_preamble(nc):
    """Move the ~11us GPSIMD library IRAM load to the very start of the Pool
    stream (before the framework init call + all-engine barrier), and the
    idx/w input DMAs into the entry block, so both overlap engine init
    instead of serializing after it."""
    from concourse import mybir

    entry = nc.main_func.blocks[0]
    il = entry.instructions
    reload_ = next(
        i for i in il if type(i).__name__ == "InstPseudoReloadLibraryIndex"
    )
    il.remove(reload_)
    il.insert(0, reload_)

    body = nc.main_func.blocks[1]
    bl = body.instructions
    dmas = [i for i in bl if isinstance(i, mybir.InstDMACopy)][:2]
    # the first two DMA copies are the idx_sb / w_sb input loads
    pos = 2  # right after [reload, InstCall]
    for d in dmas:
        bl.remove(d)
        il.insert(pos, d)
        pos += 1


def get_nc():
    if "nc" not in _NC_CACHE:
        _NC_CACHE["nc"] = _build_nc()
    return _NC_CACHE["nc"]


def _wrap_idx(flat):
    """SWDGE index layout: unwrapped[i] = wrapped[i % 16, i // 16], replicated
    across the 8 GPSIMD 16-partition groups."""
    n = flat.shape[0]
    wrapped = flat.reshape(n // 16, 16).T  # [16, n/16]
    return np.tile(wrapped, (8, 1))        # [128, n/16]


def make_inputs(x, hash_idx, weights):
    """Host-side sharding/layout prep. Returns in_maps for the 8 cores."""
    x = np.asarray(x, dtype=np.float32)
    hash_idx = np.asarray(hash_idx)
    weights = np.asarray(weights, dtype=np.float32)

    xtr = np.ascontiguousarray(x.reshape(B, CHW).T)  # [CHW, B] f32

    # idxmat[ck, hw] with ck = c*9 + k
    idxmat = (
        hash_idx.reshape(C, HWT, K).transpose(0, 2, 1).reshape(CK, HWT)
    ).astype(np.int16)

    # weights, stationary layout: [contraction partitions, 64 kp]
    w_sb = np.zeros((128, 256), dtype=np.float32)
    w_sb[:, 0:64] = weights[:, 0:128].T
    w_sb[:, 64:128] = weights[:, 128:256].T
    w_sb[:, 128:192] = np.tile(weights[:, 256:288].T, (4, 1))
    # m4=3 special stationary: rows 64..95 (m4=2 data) zero, 96..127 real
    w_sb[96:128, 192:256] = weights[:, 256:288].T

    # column order inside a core: col = m4*2048 + q*64 + b ; hw_local = 4q+m4
    j_arange = np.arange(HWC)
    hw_of_slot = 4 * (j_arange % 32) + j_arange // 32  # slot j -> hw_local

    in_maps = []
    for m in range(NCORES):
        sub = idxmat[:, m * HWC:(m + 1) * HWC]  # [288, 128] int16
        # chunks 0/1: i = j*128 + p (slot-major, ck-local minor)
        c0 = np.ascontiguousarray(sub[0:128, hw_of_slot].T).reshape(-1)
        c1 = np.ascontiguousarray(sub[128:256, hw_of_slot].T).reshape(-1)
        # chunk 2: i = q*128 + m4*32 + ck_local ; partition = 32*m4+ck
        #   value  = idx[256+ck, hw = 4q+m4]
        sub2 = sub[256:288, :]                     # [32, 128]
        arr2 = sub2.T.reshape(32, 4, 32)           # [q, m4, ck]
        c2 = np.ascontiguousarray(arr2).reshape(-1)
        idx_all = np.concatenate(
            [_wrap_idx(c0), _wrap_idx(c1), _wrap_idx(c2)], axis=1
        )
        idx_all = np.ascontiguousarray(idx_all, dtype=np.int16)
        in_maps.append({"xt": xtr, "idx": idx_all, "w": w_sb})
    return in_maps


def assemble_output(shards):
    """shards[m]: [KP, 8192] in (m4, q, b) column order -> [B, KP, H, W]."""
    out = np.empty((B, KP, HWT), dtype=np.float32)
    for m in range(NCORES):
        sh = np.asarray(shards[m]).reshape(KP, 4, 32, B)  # [kp, m4, q, b]
        blk = sh.transpose(3, 0, 2, 1).reshape(B, KP, HWC)  # hw = 4q + m4
        out[:, :, m * HWC:(m + 1) * HWC] = blk
    return out.reshape(B, KP, H, W)


def kernel(x, hash_idx, weights):
    import time

    from concourse.bass_utils import run_bass_kernel_spmd

    in_maps = make_inputs(x, hash_idx, weights)
    last_err = None
    for attempt in range(4):
        try:
            nc = get_nc()
            res = run_bass_kernel_spmd(nc, in_maps, list(range(NCORES)))
            shards = [res.results[m]["out"] for m in range(NCORES)]
            return assemble_output(shards)
        except Exception as e:  # transient NRT/device errors — retry
            last_err = e
            _NC_CACHE.clear()  # rebuild graph/executable on retry
            time.sleep(5.0 * (attempt + 1))
    raise last_err

